# revision 1
# baseline (speedup 1.0000x reference)
"""Causal segment-masked depthwise conv (K=5) + pointwise conv, 8-core SPMD.

Strategy (bf16 data path):
  Host: pack each batch row's covered segments into one gap-free global
  stream (T = 32768 = 8 cores x 4096), split evenly with a 4-element halo,
  pre-transpose to [C, stream] and cast to bf16.  Cross-run tap leakage
  (the stream has no zero gaps between runs) is fixed by a tiny sparse
  host-side correction at the first 4 positions of each run.  b_dw is
  folded into an effective output bias b_eff = b_pw + w_pw @ b_dw so the
  device conv is bias-free.
  Device per core: 4 superblocks of 1024 cols.  Depthwise conv for
  channel chunks 0,1 on DVE (tensor_scalar seed at 4x + 4 taps via
  scalar_tensor_tensor at 1x); chunks 2,3 (and chunk 1 of the last
  superblock) on PE as diag-matmul pairs into 2-bank PSUM with one
  1024-wide ACT fp32->bf16 copy per chunk-superblock.  DVE runs the last
  superblock's chunk-0 chain first so the pipeline has no tail stall.
  Pointwise: per superblock 32 bf16 matmuls (stationary w_pw^T chunk
  feeding both 512-blocks), PSUM fp32, ACT adds b_eff over 1024 cols and
  casts to bf16, 4 batched stores.  Host upcasts + scatters + corrects.
"""

import sys

sys.path.insert(0, "/opt/trn_rl_repo")

import numpy as np
import ml_dtypes

BF16 = ml_dtypes.bfloat16

B, L, C, K, S = 8, 4096, 512, 5, 8
NCORES = 8
CCH = C // 128          # 4 channel chunks
Q = 4096                # stream cols per core
NSB = 4                 # 1024-wide superblocks per core
SBW = 1024
SBPAD = 1032            # 4 halo + 1024 + 4 pad
BLK = 512
NPE = 2                 # chunks 2,3 conv'd on PE everywhere
DVE_CH = {              # conv chunks owned by DVE, per superblock
    0: (0, 1),
    1: (0, 1),
    2: (0, 1),
    3: (0,),            # sb3 chunk 1 moves to PE to kill the tail stall
}

_cached = {}


def _build_nc():
    import concourse.mybir as mybir
    from concourse import bacc
    from concourse.tile import TileContext

    f32 = mybir.dt.float32
    bf16 = mybir.dt.bfloat16
    Alu = mybir.AluOpType

    nc = bacc.Bacc(num_swdge_queues=1)
    xin_d = nc.declare_dram_parameter("xin", [NSB, 128, CCH, SBPAD], bf16, isOutput=False)
    # cst: wdiag for DVE chunks 0,1 (2*K cols) then beff (CCH cols), fp32
    cst_d = nc.declare_dram_parameter("cst", [128, 2 * K + CCH], f32, isOutput=False)
    # diag taps for all 4 chunks (PE conv may touch chunk 1 on sb3)
    diag_d = nc.declare_dram_parameter("diag", [128, CCH, K, 128], bf16, isOutput=False)
    wpwt_d = nc.declare_dram_parameter("wpwt", [128, CCH, CCH, 128], bf16, isOutput=False)
    out_d = nc.declare_dram_parameter("out", [NSB, 128, CCH, SBW], bf16, isOutput=True)

    with TileContext(nc) as tc:
        with (
            tc.tile_pool(name="consts", bufs=1) as cpool,
            tc.tile_pool(name="xin", bufs=4) as xin_pool,
            tc.tile_pool(name="dwt", bufs=4) as dwt_pool,
            tc.tile_pool(name="outsb", bufs=2) as ob_pool,
            tc.tile_pool(name="dwps", bufs=2, space="PSUM") as dw_psum,
            tc.tile_pool(name="outps", bufs=3, space="PSUM") as out_psum,
        ):
            # consts + sb1 on the ACT ring (stores come later); remaining
            # x loads on the sync ring, sb3 first (both engines start on it)
            cst = cpool.tile([128, 2 * K + CCH], f32)
            nc.scalar.dma_start(out=cst[:], in_=cst_d[:])
            wdiag = cst[:, 0 : 2 * K]
            beff = cst[:, 2 * K : 2 * K + CCH]
            wpwt = cpool.tile([128, CCH, CCH, 128], bf16)
            nc.scalar.dma_start(out=wpwt[:], in_=wpwt_d[:])
            diag = cpool.tile([128, CCH, K, 128], bf16)

            xts = [None] * NSB
            t3 = xin_pool.tile([128, CCH, SBPAD], bf16, tag="xin", name="xin3")
            nc.sync.dma_start(out=t3[:], in_=xin_d[3])
            nc.sync.dma_start(out=diag[:], in_=diag_d[:])
            xts[3] = t3
            for sb in (0, 2):
                t = xin_pool.tile([128, CCH, SBPAD], bf16, tag="xin", name=f"xin{sb}")
                nc.sync.dma_start(out=t[:], in_=xin_d[sb])
                xts[sb] = t
            t = xin_pool.tile([128, CCH, SBPAD], bf16, tag="xin", name="xin1")
            nc.scalar.dma_start(out=t[:], in_=xin_d[1])
            xts[1] = t

            # PE warm-up on a memset tile (no DMA dependency): lift the
            # HAM clock gate and bridge until the first conv data lands
            warm_t = cpool.tile([128, BLK], bf16)
            nc.vector.memset(warm_t[:], 0.0)
            warm_ps = out_psum.tile([128, 2, BLK], f32, tag="outps", name="warm_ps")
            for wi in range(24):
                nc.tensor.matmul(
                    warm_ps[:, 0, :],
                    lhsT=warm_t[:, 0:128],
                    rhs=warm_t[:],
                    start=(wi == 0),
                    stop=(wi == 23),
                )

            # per (sb, chunk) dw tiles so dependency tracking stays clean
            dwts = [
                [
                    dwt_pool.tile([128, SBW], bf16, tag=f"dwt{j}", name=f"dwt{j}_{sb}")
                    for j in range(CCH)
                ]
                for sb in range(NSB)
            ]

            def conv_dve(sb, j):
                xtv = xts[sb]
                sl = dwts[sb][j][:]
                nc.vector.tensor_scalar_mul(
                    sl, xtv[:, j, 0:SBW], wdiag[:, j * K : j * K + 1]
                )
                for k in range(1, K):
                    nc.vector.scalar_tensor_tensor(
                        sl,
                        xtv[:, j, k : k + SBW],
                        wdiag[:, j * K + k : j * K + k + 1],
                        sl,
                        op0=Alu.mult,
                        op1=Alu.add,
                    )

            def conv_pe(sb, j):
                # depthwise for chunk j over the whole superblock: per tap
                # one stationary feeds both 512-halves (two PSUM singles)
                xtv = xts[sb]
                ps = [
                    dw_psum.tile([128, BLK], f32, tag="dwps", name=f"ps{j}_{sb}_{h}")
                    for h in range(2)
                ]
                for k in range(K):
                    for h in range(2):
                        nc.tensor.matmul(
                            ps[h][:],
                            lhsT=diag[:, j, k, :],
                            rhs=xtv[:, j, h * BLK + k : h * BLK + k + BLK],
                            start=(k == 0),
                            stop=(k == K - 1),
                        )
                for h in range(2):
                    nc.scalar.copy(dwts[sb][j][:, h * BLK : (h + 1) * BLK], ps[h][:])

            def pointwise(sb):
                dt = dwts[sb]
                ob = ob_pool.tile([128, CCH, SBW], bf16, tag="outsb", name=f"ob{sb}")
                for dch in range(CCH):
                    po = out_psum.tile(
                        [128, 2, BLK], f32, tag="outps", name=f"po{dch}_{sb}"
                    )
                    for j in range(CCH):
                        for h in range(2):
                            nc.tensor.matmul(
                                po[:, h, :],
                                lhsT=wpwt[:, j, dch, :],
                                rhs=dt[j][:, h * BLK : (h + 1) * BLK],
                                start=(j == 0),
                                stop=(j == CCH - 1),
                            )
                    nc.scalar.add(ob[:, dch, :], po[:], beff[:, dch : dch + 1])
                    if dch % 2 == 1:
                        # store in halves so the final DMA trails less
                        nc.scalar.dma_start(
                            out=out_d[sb, :, dch - 1 : dch + 1, :],
                            in_=ob[:, dch - 1 : dch + 1, :],
                        )

            # DVE program order: sb3 chunk-0 chain first, then sb0..sb2
            conv_dve(3, 0)
            for sb in range(3):
                for j in DVE_CH[sb]:
                    conv_dve(sb, j)

            # PE program order: conv sb3 (its pw only needs the early DVE
            # chain), conv sb0, pw sb3, then steady state one sb ahead
            for j in (1, 2, 3):
                conv_pe(3, j)
            for j in (2, 3):
                conv_pe(0, j)
            pointwise(3)
            for j in (2, 3):
                conv_pe(1, j)
            pointwise(0)
            for j in (2, 3):
                conv_pe(2, j)
            pointwise(1)
            pointwise(2)

    nc.finalize()
    return nc


def _get_nc():
    if "nc" not in _cached:
        _cached["nc"] = _build_nc()
    return _cached["nc"]


def _analyze(segment_boundaries):
    starts = segment_boundaries[..., 0].astype(np.int64)  # [B,S]
    ends = segment_boundaries[..., 1].astype(np.int64)
    pos = np.arange(L)
    in_seg = (pos[None, None, :] >= starts[..., None]) & (
        pos[None, None, :] < ends[..., None]
    )  # [B,S,L]
    covered = in_seg.any(axis=1)
    seg_id = np.where(covered, in_seg.argmax(axis=1), -1)  # [B,L]
    return covered, seg_id


def kernel(x, segment_boundaries, w_dw, b_dw, w_pw, b_pw):
    from concourse.bass_utils import run_bass_kernel_spmd

    x = np.asarray(x, dtype=np.float32)
    sb_in = np.asarray(segment_boundaries)
    w_dw = np.asarray(w_dw, dtype=np.float32)
    b_dw = np.asarray(b_dw, dtype=np.float32)
    w_pw = np.asarray(w_pw, dtype=np.float32)
    b_pw = np.asarray(b_pw, dtype=np.float32)

    covered, seg_id = _analyze(sb_in)

    # ---- gap-free run decomposition ----
    runs = []  # (b, s, e, p0) with p0 = stream offset
    pieces = []
    src_b_parts = []
    src_l_parts = []
    p0 = 0
    for b in range(B):
        sid = seg_id[b]
        change = np.nonzero(np.diff(sid) != 0)[0] + 1
        bounds = np.concatenate([[0], change, [L]])
        for s, e in zip(bounds[:-1], bounds[1:]):
            if sid[s] < 0:
                continue
            runs.append((b, int(s), int(e), p0))
            pieces.append(x[b, s:e])
            src_b_parts.append(np.full(e - s, b, np.int64))
            src_l_parts.append(np.arange(s, e, dtype=np.int64))
            p0 += e - s
    if pieces:
        stream = np.concatenate(pieces, axis=0)
        src_b = np.concatenate(src_b_parts)
        src_l = np.concatenate(src_l_parts)
    else:
        stream = np.zeros((0, C), np.float32)
        src_b = np.zeros(0, np.int64)
        src_l = np.zeros(0, np.int64)
    T = stream.shape[0]
    Qc = -(-T // NCORES) if T else 1
    assert Qc <= Q, f"stream quota {Qc} too large"

    # ---- per-core inputs ----
    wdiag = np.ascontiguousarray(
        w_dw.reshape(CCH, 128, K)[:2].transpose(1, 0, 2).reshape(128, 2 * K)
    )
    beff_full = b_pw + w_pw @ b_dw                      # [C]
    beffr = np.ascontiguousarray(beff_full.reshape(CCH, 128).T)
    cst = np.concatenate([wdiag, beffr], axis=1).astype(np.float32)
    diag = np.zeros((128, CCH, K, 128), np.float32)
    for j in range(CCH):
        for k in range(K):
            np.fill_diagonal(diag[:, j, k, :], w_dw[j * 128 : (j + 1) * 128, k])
    diag = diag.astype(BF16)
    wpwt = np.ascontiguousarray(
        w_pw.reshape(CCH, 128, CCH, 128).transpose(3, 2, 0, 1)
    ).astype(BF16)

    # transposed bf16 stream with 4 zero cols in front
    streamT = np.zeros((C, 4 + T), dtype=BF16)
    streamT[:, 4:] = stream.T.astype(BF16)

    in_maps = []
    spans = []
    for i in range(NCORES):
        lo, hi = i * Qc, min((i + 1) * Qc, T)
        lo = min(lo, T)
        spans.append((lo, hi))
        xin = np.zeros((NSB, 128, CCH, SBPAD), dtype=BF16)
        for sbi in range(NSB):
            a = lo + sbi * SBW            # first needed stream col minus 4
            w = min(SBW + 4, 4 + T - a)
            if w <= 0:
                continue
            blkdat = streamT[:, a : a + w]  # [C, w]
            xin[sbi, :, :, :w] = blkdat.reshape(CCH, 128, w).transpose(1, 0, 2)
        in_maps.append({"xin": xin, "cst": cst, "diag": diag, "wpwt": wpwt})

    nc = _get_nc()
    res = run_bass_kernel_spmd(nc, in_maps, list(range(NCORES)))

    # ---- gather (device out is [NSB, 128, CCH, SBW] block-packed) ----
    so_out = np.zeros((T, C), np.float32)
    for i, (lo, hi) in enumerate(spans):
        if hi > lo:
            full = (
                res.results[i]["out"]
                .astype(np.float32)
                .transpose(0, 3, 2, 1)
                .reshape(NSB * SBW, C)
            )
            so_out[lo:hi] = full[: hi - lo]
    out = np.zeros((B, L, C), np.float32)
    out[src_b, src_l] = so_out

    # ---- sparse correction at the first 4 positions of each run ----
    fix_b, fix_l, fix_delta = [], [], []
    for (b, s, e, p0r) in runs:
        n = e - s
        for q in range(min(K - 1, n)):
            l = s + q
            t = p0r + q
            acc = np.zeros(C, np.float32)
            hit = False
            for d in range(q + 1, K):
                v_dev = stream[t - d] if t - d >= 0 else None
                l2 = l - d
                v_ref = (
                    x[b, l2]
                    if (l2 >= 0 and seg_id[b, l2] == seg_id[b, l])
                    else None
                )
                if v_dev is None and v_ref is None:
                    continue
                diff = (v_ref if v_ref is not None else 0.0) - (
                    v_dev if v_dev is not None else 0.0
                )
                acc += w_dw[:, K - 1 - d] * diff
                hit = True
            if hit:
                fix_b.append(b)
                fix_l.append(l)
                fix_delta.append(acc)
    if fix_b:
        deltas = np.stack(fix_delta) @ w_pw.T
        out[np.array(fix_b), np.array(fix_l)] += deltas

    return out

